# revision 37
# baseline (speedup 1.0000x reference)
"""LocallyConnected2d (64,64,32,32) x (1,64,64,32,32,9) -> (64,64,32,32) on 8 trn2 cores.

Strategy (v3: x-stationary, parity-alternating PE column groups)
----------------------------------------------------------------
Spatial sharding over output rows: core i computes output rows [4i, 4i+4).

The op is per-location GEMMs out[b, o] += sum_{c,kh,kw} x(c, X+kh, Y+kw) W(o, c, kh, kw).
W-stationary is LDWEIGHTS-fill bound (fill ~ cols/1.2GHz, every W element
loaded once -> ~41us/core). So X is the stationary operand and W streams:

  stationary S(r, j) = [K=128, M=64]: partitions 0-63 = x(c, row r, col j),
  partitions 64-127 = x(c, row r+1, col j) (row-shifted copy B); M = batch.
  moving = W col-blocks [K, N]; out[m=batch, n=(loc, oc)] accumulates in PSUM.

Tap coverage per loc (X, Y):
  - dominoes (kh=0,1) x kw: full-K matmuls at sweep r=X, blocks j=Y+kw.
  - singles (kh=2) x kw: K=64 matmuls: X=0 upper/X=1 lower at sweep r=2,
    X=2 upper/X=3 lower at sweep r=4; upper and lower share W columns.

PSUM layout puts loc parity (Y%2) on the partition half, so per block the
two matmuls (even locs -> partitions 0-63, odd -> 64-127) target opposite
PE column groups: each matmul's stationary fill overlaps the previous
matmul's stream (the PE only overlaps LDWEIGHTS with in-flight MATMULs on
non-conflicting array tiles). Without this, every fill serializes (~53ns x
~550 fills).

Each sweep runs as passes (dominoes, singles-upper, singles-lower) with W
laid out in exact consumption order; banks flush (fp32->fp16 cast + DMA
out) as soon as their output row completes, spreading the output DMA.
Bias is folded in by initializing each psum bank with a K=1 matmul.

DMA floor ~36.5us/core (W 9.4MB + x 1.7MB + out 1MB fp16 at ~330GB/s);
tensor ~32us hides under it. All W chunk DMAs are issued up front on two
queues into dedicated SBUF buffers (no ring reuse), x lands first.
"""

import numpy as np

N_B, C, H, W_W, O = 64, 64, 32, 32, 64
NCORES = 8
RPC = H // NCORES            # 4 output rows per core
BAND = RPC + 2               # 6 padded input rows per core
WP = W_W + 2                 # 34 padded width
ROWF = WP * N_B              # 2176 elems per band row (w, b), b innermost
XFREE = BAND * ROWF          # 13056
WCOLS = 36864                # total W stream columns per core
CHUNK_TARGET = 2048          # ~4KB/partition per W chunk DMA

COMPUTE_NP = np.float16

_CACHE = {}


def _locs(j):
    return [y for y in (j - 2, j - 1, j) if 0 <= y < W_W]


def _pgroups(j):
    """Per block: even-parity locs first, then odd (fixed col-group order),
    each split at psum half-bank boundaries (Y//8). Returns list of loc-runs."""
    out = []
    for p in (0, 1):
        run = []
        for y in _locs(j):
            if y % 2 != p:
                continue
            if run and (run[-1] // 8) == (y // 8):
                run.append(y)
            else:
                if run:
                    out.append(run)
                run = [y]
        if run:
            out.append(run)
    return out


def _mm_rec(kind, r, j, run, X, plo, wc0):
    # psum: bank = 2X + Y//16, partitions by loc parity, cols by
    # ((Y%16)//2)*64 + oc; "tile" ids the quarter-piece 4X + Y//8 used for
    # flush scheduling (runs never cross Y//8, so pieces flush independently)
    y0 = run[0]
    return dict(
        kind=kind, r=r, j=j,
        plo=plo, phi=plo + (128 if kind == "dom" else 64),
        wc0=wc0, n=64 * len(run),
        bank=2 * X + y0 // 16,
        tile=4 * X + y0 // 8,
        poff=64 * (y0 % 2),
        pc0=((y0 % 16) // 2) * 64,
        ys=list(run),
    )


# Per sweep r: dominoes ("dom", X, upper-tap-base) and paired singles
# ("sng", X_up, X_lo).  Even X rows use kh01 dominoes (tap base 0) + kh2
# single; odd X rows use kh12 dominoes (tap base 3) + kh0 single.  The
# sweep-1/3 singles pair (X odd, kh0) on the upper half with (X-1, kh2)
# on the lower half, sharing W columns.  Last sweep is dominoes (fast
# consumers) so the tensor tail after the final W bytes is short.
_SWEEPS = [
    (0, [("dom", 0, 0)]),
    (1, [("sng", 1, 0)]),
    (2, [("dom", 1, 3), ("dom", 2, 0)]),
    (3, [("sng", 3, 2)]),
    (4, [("dom", 3, 3)]),
]
_INIT_BANKS = {0: [0, 1], 1: [2, 3], 2: [4, 5], 3: [6, 7]}
# output row X_r completes during sweep r; its psum tile 4X+q (locs
# Y in [8q, 8q+8)) sees its last write at block j = 8q+9, so it can flush
# (cast + DMA out) while later blocks still run — separate tiles mean no
# write-after-read hazard.
_SWEEP_DONE_X = {1: 0, 2: 1, 3: 2, 4: 3}
_FLUSH_AT_J = {9: 0, 17: 1, 25: 2}


def plan_core():
    """Layout plan shared by builder and host packer.

    Returns (wblocks, prog, chunks):
      wblocks: ordered 64-col W blocks: (j, y, Xu, tu, Xl, tl): upper half
               carries W[X=Xu, y, tap tu+(j-y)], lower W[Xl, y, tl+(j-y)].
      prog: ordered program entries: MM dicts, init and flush markers.
      chunks: (col_start, col_end) W chunk DMA boundaries.
    """
    wblocks = []
    prog = []
    col = 0
    chunks = []
    chunk_start = 0

    def close_chunk():
        nonlocal chunk_start
        if col > chunk_start:
            chunks.append((chunk_start, col))
            chunk_start = col

    for r, passes in _SWEEPS:
        if r in _INIT_BANKS:
            prog.append(dict(kind="init", banks=_INIT_BANKS[r]))
        for j in range(WP):
            for kind, Xa, tb in passes:
                if kind == "dom":
                    for run in _pgroups(j):
                        for y in run:
                            wblocks.append((j, y, Xa, tb, Xa, tb + 3))
                        prog.append(_mm_rec("dom", r, j, run, Xa, 0, col))
                        col += 64 * len(run)
                else:
                    XU, XL = Xa, tb
                    for run in _pgroups(j):
                        for y in run:
                            wblocks.append((j, y, XU, 0, XL, 6))
                        prog.append(_mm_rec("sng", r, j, run, XU, 0, col))
                        prog.append(_mm_rec("sng", r, j, run, XL, 64, col))
                        col += 64 * len(run)
            if r in _SWEEP_DONE_X and j in _FLUSH_AT_J:
                prog.append(dict(
                    kind="flush", tile=4 * _SWEEP_DONE_X[r] + _FLUSH_AT_J[j]
                ))
            target = CHUNK_TARGET // 2 if r == 4 else CHUNK_TARGET
            if col - chunk_start >= target:
                close_chunk()
        if r in _SWEEP_DONE_X:
            prog.append(dict(kind="flush", tile=4 * _SWEEP_DONE_X[r] + 3))
    close_chunk()
    assert col == WCOLS, col
    return wblocks, prog, chunks


def _mybir_dt(np_dt):
    import concourse.mybir as mybir
    import ml_dtypes

    if np_dt == np.float16:
        return mybir.dt.float16
    if np_dt == np.float32:
        return mybir.dt.float32
    if np_dt == ml_dtypes.bfloat16:
        return mybir.dt.bfloat16
    raise ValueError(np_dt)


def build_nc(compute_np=None):
    """Build the (single-program) Bass kernel; same NEFF runs on all 8 cores."""
    import concourse.bass as bass  # noqa: F401
    import concourse.mybir as mybir
    import concourse.tile as tile
    from concourse import bacc
    from contextlib import ExitStack

    cdt = _mybir_dt(compute_np or COMPUTE_NP)
    f32 = mybir.dt.float32

    _, prog, chunks = plan_core()

    nc = bacc.Bacc("TRN2", target_bir_lowering=False, debug=False)

    x_dram = nc.dram_tensor("xb", [64, XFREE], cdt, kind="ExternalInput")
    w_dram = nc.dram_tensor("wp", [128, WCOLS], cdt, kind="ExternalInput")
    b_dram = nc.dram_tensor("bp", [1, 16 * 512], cdt, kind="ExternalInput")
    o_dram = nc.dram_tensor("out", [8, 128, 512], cdt, kind="ExternalOutput")

    with ExitStack() as ctx:
        tc = ctx.enter_context(tile.TileContext(nc))
        const = ctx.enter_context(tc.tile_pool(name="const", bufs=1))
        wpool = ctx.enter_context(tc.tile_pool(name="wpool", bufs=1))
        ppool = ctx.enter_context(tc.tile_pool(name="ppool", bufs=1, space="PSUM"))
        spool = ctx.enter_context(tc.tile_pool(name="spool", bufs=1))

        xsb = const.tile([128, XFREE], cdt)
        bias_sb = const.tile([1, 16 * 512], cdt)
        ones_sb = const.tile([1, 64], cdt)

        nc.gpsimd.memset(ones_sb[:], 1.0)
        nc.gpsimd.dma_start(bias_sb[:], b_dram.ap()[:, :])

        # x copy A (partitions 0-63): rows 0-2 first so sweeps 0-1 unblock;
        # x must lead the W stream on its queue or copy B (and the first
        # matmuls) wait behind megabytes of W.
        nc.sync.dma_start(xsb[0:64, 0 : 3 * ROWF], x_dram.ap()[:, 0 : 3 * ROWF])
        nc.sync.dma_start(
            xsb[0:64, 3 * ROWF : XFREE], x_dram.ap()[:, 3 * ROWF : XFREE]
        )

        # all W chunk DMAs issued up front, alternating two queues
        wtiles = []
        for ci, (c0, c1) in enumerate(chunks):
            wt = wpool.tile([128, c1 - c0], cdt, name=f"wt{ci}")
            eng = nc.scalar if ci % 2 == 0 else nc.gpsimd
            eng.dma_start(wt[:], w_dram.ap()[:, c0:c1])
            wtiles.append(wt)
        # copy B = row-shifted copy A on partitions 64-127 (idle vector engine)
        nc.vector.tensor_copy(
            xsb[64:128, 0 : 2 * ROWF], xsb[0:64, ROWF : 3 * ROWF]
        )
        nc.vector.tensor_copy(
            xsb[64:128, 2 * ROWF : 5 * ROWF], xsb[0:64, 3 * ROWF : XFREE]
        )

        ps = [ppool.tile([128, 512], f32, name=f"ps{b}") for b in range(8)]

        def bias_init(bank):
            # psum[p, n] = bias[n] for all partitions p (K=1 ones stationary);
            # alternate halves so fills alternate PE col groups too
            for half in range(2):
                b0 = (2 * bank + half) * 512
                nc.tensor.matmul(
                    ps[bank][64 * half : 64 * half + 64, 0:512],
                    ones_sb[0:1, 0:64],
                    bias_sb[0:1, b0 : b0 + 512],
                    start=True,
                    stop=False,
                    skip_group_check=True,
                )

        def chunk_of(c):
            for ci, (c0, c1) in enumerate(chunks):
                if c0 <= c < c1:
                    return ci, c - c0
            raise ValueError(c)

        x4 = xsb[:].rearrange("p (h w b) -> p h w b", h=BAND, w=WP)

        def flush(tile_idx):
            bank, hb = tile_idx // 2, tile_idx % 2
            c0 = hb * 256
            stg = spool.tile([128, 256], cdt, name=f"stg{tile_idx}")
            nc.vector.tensor_copy(stg[:], ps[bank][:, c0 : c0 + 256])
            # sync queue only: x is long done; a W queue would FIFO the out
            # behind megabytes of still-queued weight descriptors
            nc.sync.dma_start(o_dram.ap()[bank][:, c0 : c0 + 256], stg[:])

        for m in prog:
            if m["kind"] == "flush":
                flush(m["tile"])
                continue
            if m["kind"] == "init":
                for bank in m["banks"]:
                    bias_init(bank)
                continue
            ci, loc0 = chunk_of(m["wc0"])
            assert m["wc0"] + m["n"] <= chunks[ci][1]
            wt = wtiles[ci]
            plo, phi = m["plo"], m["phi"]
            stat = x4[plo:phi, m["r"], m["j"], :]
            mov = wt[plo:phi, loc0 : loc0 + m["n"]]
            out = ps[m["bank"]][
                m["poff"] : m["poff"] + 64, m["pc0"] : m["pc0"] + m["n"]
            ]
            nc.tensor.matmul(
                out, stat, mov, start=False, stop=False, skip_group_check=True
            )
    nc.compile()
    return nc


def pack_inputs(x, weight, bias, compute_np=None):
    """Full fp32 inputs -> list of 8 per-core input dicts (device layouts)."""
    cnp = compute_np or COMPUTE_NP
    x = np.asarray(x)
    w5 = np.asarray(weight)[0]        # (o, c, X, Y, k)
    b3 = np.asarray(bias)[0]          # (o, X, Y)

    xp = np.pad(x, ((0, 0), (0, 0), (1, 1), (1, 1)))  # (b, c, 34, 34)
    wblocks, _, _ = plan_core()

    # (X, Y, k, c, o): each 64-col W block is w5t[X, y, k] = [c, o]
    w5t = np.ascontiguousarray(w5.transpose(2, 3, 4, 1, 0)).astype(cnp)

    in_maps = []
    for i in range(NCORES):
        band = xp[:, :, RPC * i : RPC * i + BAND, :]          # (b, c, 6, 34)
        xb = np.ascontiguousarray(band.transpose(1, 2, 3, 0)) # (c, 6, 34, b)
        xb = xb.astype(cnp).reshape(64, XFREE)

        wp = np.empty((128, WCOLS), dtype=cnp)
        col = 0
        for (j, y, Xu, tu, Xl, tl) in wblocks:
            wp[0:64, col : col + 64] = w5t[4 * i + Xu, y, tu + (j - y)]
            wp[64:128, col : col + 64] = w5t[4 * i + Xl, y, tl + (j - y)]
            col += 64
        assert col == WCOLS

        # bias: [1, 16*512]: (bank, half) -> 8 locs x 64 oc
        # bank = 2X + Y//16 ; half = Y%2 ; col = ((Y%16)//2)*64 + oc
        bp = np.empty((1, 16 * 512), dtype=cnp)
        for bank in range(8):
            X = bank // 2
            for half in range(2):
                ys = np.arange(16 * (bank % 2) + half, 16 * (bank % 2) + 16, 2)
                blk = b3[:, 4 * i + X, ys]                    # (o, 8)
                bp[0, (2 * bank + half) * 512 : (2 * bank + half + 1) * 512] = (
                    blk.T.reshape(-1).astype(cnp)
                )

        in_maps.append({"xb": xb, "wp": wp, "bp": bp})
    return in_maps


def unpack_output(core_outs):
    """8 per-core [8, 128, 512] arrays -> full (64, 64, 32, 32) output."""
    arr = np.stack(core_outs)                      # (core, bank, p, col)
    # bank = 2X + hb ; p = 64*(Y%2) + b ; col = ((Y%16)//2)*64 + o
    arr = arr.reshape(8, 4, 2, 2, 64, 8, 64)       # core X hb par b q o
    # Y = hb*16 + q*2 + par
    out = arr.transpose(4, 6, 0, 1, 2, 5, 3)       # b o core X hb q par
    return np.ascontiguousarray(
        out.reshape(64, 64, 32, 32), dtype=np.float32
    )


def run_on_device(in_maps, trace=False, compute_np=None, **kwargs):
    from concourse import bass_utils

    key = ("nc", np.dtype(compute_np or COMPUTE_NP).name)
    if key not in _CACHE:
        _CACHE[key] = build_nc(compute_np)
    nc = _CACHE[key]
    res = bass_utils.run_bass_kernel_spmd(
        nc, in_maps, core_ids=list(range(NCORES)), trace=trace, **kwargs
    )
    return res


def kernel(x, weight, bias):
    in_maps = pack_inputs(x, weight, bias)
    res = run_on_device(in_maps)
    return unpack_output([r["out"] for r in res.results])


# revision 42
# speedup vs baseline: 1.0680x; 1.0680x over previous
"""LocallyConnected2d (64,64,32,32) x (1,64,64,32,32,9) -> (64,64,32,32) on 8 trn2 cores.

Strategy (v3: x-stationary, parity-alternating PE column groups)
----------------------------------------------------------------
Spatial sharding over output rows: core i computes output rows [4i, 4i+4).

The op is per-location GEMMs out[b, o] += sum_{c,kh,kw} x(c, X+kh, Y+kw) W(o, c, kh, kw).
W-stationary is LDWEIGHTS-fill bound (fill ~ cols/1.2GHz, every W element
loaded once -> ~41us/core). So X is the stationary operand and W streams:

  stationary S(r, j) = [K=128, M=64]: partitions 0-63 = x(c, row r, col j),
  partitions 64-127 = x(c, row r+1, col j) (row-shifted copy B); M = batch.
  moving = W col-blocks [K, N]; out[m=batch, n=(loc, oc)] accumulates in PSUM.

Tap coverage per loc (X, Y):
  - dominoes (kh=0,1) x kw: full-K matmuls at sweep r=X, blocks j=Y+kw.
  - singles (kh=2) x kw: K=64 matmuls: X=0 upper/X=1 lower at sweep r=2,
    X=2 upper/X=3 lower at sweep r=4; upper and lower share W columns.

PSUM layout puts loc parity (Y%2) on the partition half, so per block the
two matmuls (even locs -> partitions 0-63, odd -> 64-127) target opposite
PE column groups: each matmul's stationary fill overlaps the previous
matmul's stream (the PE only overlaps LDWEIGHTS with in-flight MATMULs on
non-conflicting array tiles). Without this, every fill serializes (~53ns x
~550 fills).

Each sweep runs as passes (dominoes, singles-upper, singles-lower) with W
laid out in exact consumption order; banks flush (fp32->fp16 cast + DMA
out) as soon as their output row completes, spreading the output DMA.
Bias is folded in by initializing each psum bank with a K=1 matmul.

DMA floor ~36.5us/core (W 9.4MB + x 1.7MB + out 1MB fp16 at ~330GB/s);
tensor ~32us hides under it. All W chunk DMAs are issued up front on two
queues into dedicated SBUF buffers (no ring reuse), x lands first.
"""

import numpy as np

N_B, C, H, W_W, O = 64, 64, 32, 32, 64
NCORES = 8
RPC = H // NCORES            # 4 output rows per core
BAND = RPC + 2               # 6 padded input rows per core
WP = W_W + 2                 # 34 padded width
ROWF = WP * N_B              # 2176 elems per band row (w, b), b innermost
XFREE = BAND * ROWF          # 13056
WCOLS = 36864                # total W stream columns per core
CHUNK_TARGET = 2048          # ~4KB/partition per W chunk DMA

COMPUTE_NP = np.float16

_CACHE = {}


def _locs(j):
    return [y for y in (j - 2, j - 1, j) if 0 <= y < W_W]


def _pgroups(j):
    """Per block: even-parity locs first, then odd (fixed col-group order),
    each split at psum half-bank boundaries (Y//8). Returns list of loc-runs."""
    out = []
    for p in (0, 1):
        run = []
        for y in _locs(j):
            if y % 2 != p:
                continue
            if run and (run[-1] // 16) == (y // 16):
                run.append(y)
            else:
                if run:
                    out.append(run)
                run = [y]
        if run:
            out.append(run)
    return out


def _mm_rec(kind, r, j, run, X, plo, wc0):
    # psum: bank = 2X + Y//16, partitions by loc parity, cols by
    # ((Y%16)//2)*64 + oc; "tile" ids the quarter-piece 4X + Y//8 used for
    # flush scheduling (runs never cross Y//8, so pieces flush independently)
    y0 = run[0]
    return dict(
        kind=kind, r=r, j=j,
        plo=plo, phi=plo + (128 if kind == "dom" else 64),
        wc0=wc0, n=64 * len(run),
        bank=2 * X + y0 // 16,
        tile=4 * X + y0 // 8,
        poff=64 * (y0 % 2),
        pc0=((y0 % 16) // 2) * 64,
        ys=list(run),
    )


# Per sweep r: dominoes ("dom", X, upper-tap-base) and paired singles
# ("sng", X_up, X_lo).  Even X rows use kh01 dominoes (tap base 0) + kh2
# single; odd X rows use kh12 dominoes (tap base 3) + kh0 single.  The
# sweep-1/3 singles pair (X odd, kh0) on the upper half with (X-1, kh2)
# on the lower half, sharing W columns.  Last sweep is dominoes (fast
# consumers) so the tensor tail after the final W bytes is short.
_SWEEPS = [
    (0, [("dom", 0, 0)]),
    (1, [("sng", 1, 0)]),
    (2, [("dom", 1, 3), ("dom", 2, 0)]),
    (3, [("sng", 3, 2)]),
    (4, [("dom", 3, 3)]),
]
_INIT_BANKS = {0: [0, 1], 1: [2, 3], 2: [4, 5], 3: [6, 7]}
# output row X_r completes during sweep r; its psum tile 4X+q (locs
# Y in [8q, 8q+8)) sees its last write at block j = 8q+9, so it can flush
# (cast + DMA out) while later blocks still run — separate tiles mean no
# write-after-read hazard.
_SWEEP_DONE_X = {1: 0, 2: 1, 3: 2, 4: 3}
_FLUSH_AT_J = {17: 0}


def plan_core():
    """Layout plan shared by builder and host packer.

    Returns (wblocks, prog, chunks):
      wblocks: ordered 64-col W blocks: (j, y, Xu, tu, Xl, tl): upper half
               carries W[X=Xu, y, tap tu+(j-y)], lower W[Xl, y, tl+(j-y)].
      prog: ordered program entries: MM dicts, init and flush markers.
      chunks: (col_start, col_end) W chunk DMA boundaries.
    """
    wblocks = []
    prog = []
    col = 0
    chunks = []
    chunk_start = 0

    def close_chunk():
        nonlocal chunk_start
        if col > chunk_start:
            chunks.append((chunk_start, col))
            chunk_start = col

    for r, passes in _SWEEPS:
        if r in _INIT_BANKS:
            prog.append(dict(kind="init", banks=_INIT_BANKS[r]))
        for j in range(WP):
            for kind, Xa, tb in passes:
                if kind == "dom":
                    for run in _pgroups(j):
                        for y in run:
                            wblocks.append((j, y, Xa, tb, Xa, tb + 3))
                        prog.append(_mm_rec("dom", r, j, run, Xa, 0, col))
                        col += 64 * len(run)
                else:
                    XU, XL = Xa, tb
                    for run in _pgroups(j):
                        for y in run:
                            wblocks.append((j, y, XU, 0, XL, 6))
                        prog.append(_mm_rec("sng", r, j, run, XU, 0, col))
                        prog.append(_mm_rec("sng", r, j, run, XL, 64, col))
                        col += 64 * len(run)
            if r in _SWEEP_DONE_X and j == 17:
                prog.append(dict(kind="flush", bank=2 * _SWEEP_DONE_X[r]))
            target = CHUNK_TARGET // 2 if r == 4 else CHUNK_TARGET
            if col - chunk_start >= target:
                close_chunk()
        if r in _SWEEP_DONE_X:
            prog.append(dict(kind="flush", bank=2 * _SWEEP_DONE_X[r] + 1))
    close_chunk()
    assert col == WCOLS, col
    return wblocks, prog, chunks


def _mybir_dt(np_dt):
    import concourse.mybir as mybir
    import ml_dtypes

    if np_dt == np.float16:
        return mybir.dt.float16
    if np_dt == np.float32:
        return mybir.dt.float32
    if np_dt == ml_dtypes.bfloat16:
        return mybir.dt.bfloat16
    raise ValueError(np_dt)


def build_nc(compute_np=None):
    """Build the (single-program) Bass kernel; same NEFF runs on all 8 cores."""
    import concourse.bass as bass  # noqa: F401
    import concourse.mybir as mybir
    import concourse.tile as tile
    from concourse import bacc
    from contextlib import ExitStack

    cdt = _mybir_dt(compute_np or COMPUTE_NP)
    f32 = mybir.dt.float32

    _, prog, chunks = plan_core()

    nc = bacc.Bacc("TRN2", target_bir_lowering=False, debug=False)

    x_dram = nc.dram_tensor("xb", [64, XFREE], cdt, kind="ExternalInput")
    w_dram = nc.dram_tensor("wp", [128, WCOLS], cdt, kind="ExternalInput")
    b_dram = nc.dram_tensor("bp", [1, 16 * 512], cdt, kind="ExternalInput")
    o_dram = nc.dram_tensor("out", [8, 128, 512], cdt, kind="ExternalOutput")

    with ExitStack() as ctx:
        tc = ctx.enter_context(tile.TileContext(nc))
        const = ctx.enter_context(tc.tile_pool(name="const", bufs=1))
        wpool = ctx.enter_context(tc.tile_pool(name="wpool", bufs=1))
        ppool = ctx.enter_context(tc.tile_pool(name="ppool", bufs=1, space="PSUM"))
        spool = ctx.enter_context(tc.tile_pool(name="spool", bufs=1))

        xsb = const.tile([128, XFREE], cdt)
        bias_sb = const.tile([1, 16 * 512], cdt)
        ones_sb = const.tile([1, 64], cdt)

        nc.gpsimd.memset(ones_sb[:], 1.0)
        nc.gpsimd.dma_start(bias_sb[:], b_dram.ap()[:, :])

        # x copy A (partitions 0-63): rows 0-2 first so sweeps 0-1 unblock;
        # x must lead the W stream on its queue or copy B (and the first
        # matmuls) wait behind megabytes of W.
        nc.sync.dma_start(xsb[0:64, 0 : 3 * ROWF], x_dram.ap()[:, 0 : 3 * ROWF])
        nc.sync.dma_start(
            xsb[0:64, 3 * ROWF : XFREE], x_dram.ap()[:, 3 * ROWF : XFREE]
        )

        # all W chunk DMAs issued up front, alternating two queues
        wtiles = []
        for ci, (c0, c1) in enumerate(chunks):
            wt = wpool.tile([128, c1 - c0], cdt, name=f"wt{ci}")
            eng = nc.scalar if ci % 2 == 0 else nc.gpsimd
            eng.dma_start(wt[:], w_dram.ap()[:, c0:c1])
            wtiles.append(wt)
        # copy B = row-shifted copy A on partitions 64-127 (idle vector engine)
        nc.vector.tensor_copy(
            xsb[64:128, 0 : 2 * ROWF], xsb[0:64, ROWF : 3 * ROWF]
        )
        nc.vector.tensor_copy(
            xsb[64:128, 2 * ROWF : 5 * ROWF], xsb[0:64, 3 * ROWF : XFREE]
        )

        ps = [ppool.tile([128, 512], f32, name=f"ps{b}") for b in range(8)]

        def bias_init(bank):
            # psum[p, n] = bias[n] for all partitions p (K=1 ones stationary);
            # alternate halves so fills alternate PE col groups too
            for half in range(2):
                b0 = (2 * bank + half) * 512
                nc.tensor.matmul(
                    ps[bank][64 * half : 64 * half + 64, 0:512],
                    ones_sb[0:1, 0:64],
                    bias_sb[0:1, b0 : b0 + 512],
                    start=True,
                    stop=False,
                    skip_group_check=True,
                )

        def chunk_of(c):
            for ci, (c0, c1) in enumerate(chunks):
                if c0 <= c < c1:
                    return ci, c - c0
            raise ValueError(c)

        x4 = xsb[:].rearrange("p (h w b) -> p h w b", h=BAND, w=WP)

        def flush(bank):
            stg = spool.tile([128, 512], cdt, name=f"stg{bank}")
            nc.vector.tensor_copy(stg[:], ps[bank][:])
            # sync queue only: x is long done; a W queue would FIFO the out
            # behind megabytes of still-queued weight descriptors
            nc.sync.dma_start(o_dram.ap()[bank], stg[:])

        for m in prog:
            if m["kind"] == "flush":
                flush(m["bank"])
                continue
            if m["kind"] == "init":
                for bank in m["banks"]:
                    bias_init(bank)
                continue
            ci, loc0 = chunk_of(m["wc0"])
            assert m["wc0"] + m["n"] <= chunks[ci][1]
            wt = wtiles[ci]
            plo, phi = m["plo"], m["phi"]
            stat = x4[plo:phi, m["r"], m["j"], :]
            mov = wt[plo:phi, loc0 : loc0 + m["n"]]
            out = ps[m["bank"]][
                m["poff"] : m["poff"] + 64, m["pc0"] : m["pc0"] + m["n"]
            ]
            nc.tensor.matmul(
                out, stat, mov, start=False, stop=False, skip_group_check=True
            )
    nc.compile()
    return nc


def pack_inputs(x, weight, bias, compute_np=None):
    """Full fp32 inputs -> list of 8 per-core input dicts (device layouts)."""
    cnp = compute_np or COMPUTE_NP
    x = np.asarray(x)
    w5 = np.asarray(weight)[0]        # (o, c, X, Y, k)
    b3 = np.asarray(bias)[0]          # (o, X, Y)

    xp = np.pad(x, ((0, 0), (0, 0), (1, 1), (1, 1)))  # (b, c, 34, 34)
    wblocks, _, _ = plan_core()

    # (X, Y, k, c, o): each 64-col W block is w5t[X, y, k] = [c, o]
    w5t = np.ascontiguousarray(w5.transpose(2, 3, 4, 1, 0)).astype(cnp)

    in_maps = []
    for i in range(NCORES):
        band = xp[:, :, RPC * i : RPC * i + BAND, :]          # (b, c, 6, 34)
        xb = np.ascontiguousarray(band.transpose(1, 2, 3, 0)) # (c, 6, 34, b)
        xb = xb.astype(cnp).reshape(64, XFREE)

        wp = np.empty((128, WCOLS), dtype=cnp)
        col = 0
        for (j, y, Xu, tu, Xl, tl) in wblocks:
            wp[0:64, col : col + 64] = w5t[4 * i + Xu, y, tu + (j - y)]
            wp[64:128, col : col + 64] = w5t[4 * i + Xl, y, tl + (j - y)]
            col += 64
        assert col == WCOLS

        # bias: [1, 16*512]: (bank, half) -> 8 locs x 64 oc
        # bank = 2X + Y//16 ; half = Y%2 ; col = ((Y%16)//2)*64 + oc
        bp = np.empty((1, 16 * 512), dtype=cnp)
        for bank in range(8):
            X = bank // 2
            for half in range(2):
                ys = np.arange(16 * (bank % 2) + half, 16 * (bank % 2) + 16, 2)
                blk = b3[:, 4 * i + X, ys]                    # (o, 8)
                bp[0, (2 * bank + half) * 512 : (2 * bank + half + 1) * 512] = (
                    blk.T.reshape(-1).astype(cnp)
                )

        in_maps.append({"xb": xb, "wp": wp, "bp": bp})
    return in_maps


def unpack_output(core_outs):
    """8 per-core [8, 128, 512] arrays -> full (64, 64, 32, 32) output."""
    arr = np.stack(core_outs)                      # (core, bank, p, col)
    # bank = 2X + hb ; p = 64*(Y%2) + b ; col = ((Y%16)//2)*64 + o
    arr = arr.reshape(8, 4, 2, 2, 64, 8, 64)       # core X hb par b q o
    # Y = hb*16 + q*2 + par
    out = arr.transpose(4, 6, 0, 1, 2, 5, 3)       # b o core X hb q par
    return np.ascontiguousarray(
        out.reshape(64, 64, 32, 32), dtype=np.float32
    )


def run_on_device(in_maps, trace=False, compute_np=None, **kwargs):
    from concourse import bass_utils

    key = ("nc", np.dtype(compute_np or COMPUTE_NP).name)
    if key not in _CACHE:
        _CACHE[key] = build_nc(compute_np)
    nc = _CACHE[key]
    res = bass_utils.run_bass_kernel_spmd(
        nc, in_maps, core_ids=list(range(NCORES)), trace=trace, **kwargs
    )
    return res


def kernel(x, weight, bias):
    in_maps = pack_inputs(x, weight, bias)
    res = run_on_device(in_maps)
    return unpack_output([r["out"] for r in res.results])
